# revision 10
# baseline (speedup 1.0000x reference)
"""GyroLoss Trainium2 kernel v4 — fp16 single batch, DVE/Pool co-processing,
PE column-sum reductions, early diff pieces.

Differences vs v3: one full-width batch (only 4 ACT table loads); every wide
vector stage split across DVE and Pool; ACT stages emitted as adjacent halves
so the table load overlaps the first half only; XYZ input DMA split in three
so squares start ~1.2us after launch; loss sums via PE ones-matmuls into one
[8,512] PSUM row set (per-half rows for the two Huber groups).

Host: loss = sum_groups 0.5*Sms2 + s*Saz - Sms  with ms = min(|z|,c)/c.
"""

import numpy as np
from contextlib import ExitStack

import concourse.bass as bass
import concourse.tile as tile
from concourse import mybir
from concourse.bass_utils import run_bass_kernel_spmd

F32 = mybir.dt.float32
F16 = mybir.dt.float16
AF = mybir.ActivationFunctionType
ALU = mybir.AluOpType

HUBER = 0.005
N0 = 5
W_LOSS = 1e6
PI = float(np.pi)
S_A = 6.0 / HUBER
S_B = 1.0 / HUBER
C_A = 1.0 / S_A
C_B = 1.0 / S_B
N_CORES = 8
NW = 64
T = 8192
COUNT = NW * (T - N0) * 15

EXP_BIAS = 6.103515625e-05   # 2^-14, fp16 min normal
LOG_BIAS = 0.25000003

_CACHED = {}

# PSUM colsum rows: 0 zxA 1 zxB 2 zyA 3 zyB 4 zzA 5 zzB 6 DA 7 DB


def _act_rsqrt(nc, out, in_, **kw):
    bi = nc.scalar.activation(out, in_, AF.Sqrt, **kw)
    bi.ins.func = AF.Rsqrt
    return bi


def _build_module():
    nc = bass.Bass()
    planes = nc.declare_dram_parameter("planes", [128, 12288], F16,
                                       isOutput=False)
    out_all = nc.declare_dram_parameter("out_all", [8, 1536], F32,
                                        isOutput=True)

    with ExitStack() as ctx:
        tc = ctx.enter_context(tile.TileContext(nc))
        pool = ctx.enter_context(tc.tile_pool(name="main", bufs=1))
        psum = ctx.enter_context(tc.tile_pool(name="ps", space="PSUM", bufs=1))

        def tl(n, w, dt=F16):
            return pool.tile([128, w], dt, name=n, tag=n)

        for dt, val in ((F16, EXP_BIAS), (F16, PI / 2), (F32, LOG_BIAS),
                        (F32, EXP_BIAS), (F32, PI / 2), (F16, LOG_BIAS)):
            t = pool.tile([128, 1], dt, name=f"c{dt}{val}", tag=f"c{dt}{val}")
            nc.gpsimd.memset(t[:], val)
            nc.const_aps.aps[(dt, val)] = t[:]

        EJ = []
        for r in range(8):
            e = pool.tile([128, 8], F16, name=f"ej{r}", tag=f"ej{r}")
            nc.gpsimd.memset(e[:], 0.0)
            nc.gpsimd.memset(e[:, r:r + 1], 1.0)
            EJ.append(e)

        AZS = psum.tile([8, 512], F32, name="AZS", tag="AZS")
        MSS = psum.tile([8, 512], F32, name="MSS", tag="MSS")
        M2S = psum.tile([8, 512], F32, name="M2S", tag="M2S")
        _first = {"az": True, "ms": True, "m2": True}

        def colsum(which, pt, row, ap, final=False):
            fd = ap.shape[-1]
            o = 0
            while o < fd:
                n = min(512, fd - o)
                st = _first[which]
                _first[which] = False
                nc.tensor.matmul(pt[:, 0:n], EJ[row][:], ap[:, o:o + n],
                                 start=st, stop=final and (o + n >= fd),
                                 skip_group_check=True)
                o += n

        # ---- inputs: partition-major; XYZ split into 3 stack DMAs ----
        XYZ = tl("XYZ", 4608)
        DVDP = tl("DVDP", 3072)
        AH = tl("AH", 1536)
        DVH = tl("DVH", 3072)
        dma = nc.sync.dma_start
        dma(XYZ[:, 0:1536], planes[:, 0:1536])
        dma(XYZ[:, 1536:3072], planes[:, 1536:3072])
        dma(XYZ[:, 3072:4608], planes[:, 3072:4608])
        dma(DVDP[:], planes[:, 4608:7680])
        dma(AH[:], planes[:, 7680:9216])
        dma(DVH[:], planes[:, 9216:12288])

        act = nc.scalar.activation
        v = nc.vector
        g = nc.gpsimd

        X = XYZ[:, 0:1536]
        Y = XYZ[:, 1536:3072]
        Z = XYZ[:, 3072:4608]

        # ---- exp head (start immediately after XYZ lands) ----
        sqx = tl("sqx", 1536)
        sqy = tl("sqy", 1536)
        sqz = tl("sqz", 1536)
        v.tensor_mul(sqx[:], X, X)
        g.tensor_mul(sqy[:], Y, Y)
        v.tensor_mul(sqz[:], Z, Z)
        t2a = tl("t2a", 1536)
        t2 = tl("t2", 1536)
        v.tensor_add(t2a[:], sqx[:], sqy[:])
        v.tensor_add(t2[:], t2a[:], sqz[:])
        rth = tl("rth", 1536)
        _act_rsqrt(nc, rth[:, 0:768], t2[:, 0:768], bias=EXP_BIAS)
        _act_rsqrt(nc, rth[:, 768:1536], t2[:, 768:1536], bias=EXP_BIAS)
        th = tl("th", 1536)
        v.tensor_mul(th[:, 0:768], t2[:, 0:768], rth[:, 0:768])
        v.tensor_mul(th[:, 768:1536], t2[:, 768:1536], rth[:, 768:1536])
        sh = tl("sh", 1536)
        Q = tl("Q", 6144)   # [qw | qx | qy | qz], each 1536 = [qa|qb|qc]
        for lo, hi in ((0, 768), (768, 1536)):
            act(sh[:, lo:hi], th[:, lo:hi], AF.Sin, scale=0.5)
            act(Q[:, lo:hi], th[:, lo:hi], AF.Sin, bias=PI / 2, scale=0.5)
        # diff subs fill the DVE/Pool wait on the sins; az fills ACT after
        DA = tl("DA", 1536)
        DB = tl("DB", 3072)
        v.tensor_sub(DA[:], DVDP[:, 0:1536], AH[:])
        v.tensor_sub(DB[:, 0:1536], DVDP[:, 0:1536], DVH[:, 0:1536])
        g.tensor_sub(DB[:, 1536:3072], DVDP[:, 1536:3072], DVH[:, 1536:3072])
        azA = tl("azA", 1536)
        azB = tl("azB", 3072)
        act(azA[:], DA[:], AF.Abs)
        act(azB[:, 0:1536], DB[:, 0:1536], AF.Abs)
        act(azB[:, 1536:3072], DB[:, 1536:3072], AF.Abs)
        s_ = tl("s_", 1536)
        v.tensor_mul(s_[:, 0:768], sh[:, 0:768], rth[:, 0:768])
        v.tensor_mul(s_[:, 768:1536], sh[:, 768:1536], rth[:, 768:1536])
        v.tensor_mul(Q[:, 1536:3072], s_[:], X)
        v.tensor_mul(Q[:, 3072:4608], s_[:], Y)
        g.tensor_mul(Q[:, 4608:6144], s_[:], Z)

        mA = tl("mA", 1536)
        mB = tl("mB", 3072)
        m2A = tl("m2A", 1536)
        m2B = tl("m2B", 3072)

        # ---- qmult: conj(qa) (x) [qb|qc], qa broadcast over both stacks
        def A(k):
            return (Q[:, 1536 * k:1536 * k + 512]
                    .rearrange("p (o f) -> p o f", o=1)
                    .broadcast_to([128, 2, 512]))

        def B(k):
            return Q[:, 1536 * k + 512:1536 * (k + 1)]

        names = {}

        def prod(nm, eng, i, j):
            tt = tl(nm, 1024)
            eng.tensor_mul(tt[:], A(i), B(j))
            names[nm] = tt
            return tt

        wr = tl("wr", 1024)
        vx = tl("vx", 1024)
        vy = tl("vy", 1024)
        vz = tl("vz", 1024)
        # w products first: the whole scalar log chain depends only on wr,
        # so it runs (incl. both table loads) under the 12 v-products.
        p0 = prod("p0", v, 0, 0)
        p1 = prod("p1", v, 1, 1)
        p2 = prod("p2", v, 2, 2)
        p3 = prod("p3", v, 3, 3)
        wa_ = tl("wa_", 1024)
        wb_ = tl("wb_", 1024)
        v.tensor_add(wa_[:], p0[:], p1[:])
        v.tensor_add(wb_[:], p2[:], p3[:])
        v.tensor_add(wr[:], wa_[:], wb_[:])
        # ---- log scalar chain (overlaps v products below); squares on ACT
        # keep the chain off Pool's laggy in-order queue and next to r/at
        w2 = tl("w2", 1024)
        act(w2[:], wr[:], AF.Square)
        a = tl("a", 1024)
        v.tensor_scalar(a[:], w2[:], 1.0, 0.5, ALU.min, ALU.subtract)
        asq = tl("asq", 1024, F32)
        act(asq[:], a[:], AF.Square)
        r = tl("r", 1024)
        _act_rsqrt(nc, r[:, 0:512], asq[:, 0:512], scale=-1.0, bias=LOG_BIAS)
        _act_rsqrt(nc, r[:, 512:1024], asq[:, 512:1024], scale=-1.0,
                   bias=LOG_BIAS)
        t_ = tl("t_", 1024)
        v.tensor_mul(t_[:, 0:512], a[:, 0:512], r[:, 0:512])
        v.tensor_mul(t_[:, 512:1024], a[:, 512:1024], r[:, 512:1024])
        rw = tl("rw", 1024)
        v.tensor_mul(rw[:], r[:], wr[:])   # overlaps the arctan
        at = tl("at", 1024, F32)
        act(at[:, 0:512], t_[:, 0:512], AF.Arctan)
        act(at[:, 512:1024], t_[:, 512:1024], AF.Arctan)
        pa = tl("pa", 1024)
        v.tensor_scalar(pa[:], at[:], -1.0, PI / 2, ALU.mult, ALU.add)
        gp2 = tl("gp2", 1024)
        v.tensor_mul(gp2[:], pa[:], rw[:])

        # diff pieces: data ready early; first in PE's in-order queue
        v.tensor_scalar(mA[:], azA[:], C_A, 1.0 / C_A, ALU.min, ALU.mult)
        v.tensor_scalar(mB[:], azB[:], C_B, 1.0 / C_B, ALU.min, ALU.mult)
        g.tensor_mul(m2A[:], mA[:], mA[:])
        g.tensor_mul(m2B[:], mB[:], mB[:])
        colsum("az", AZS, 6, azA[:])
        colsum("ms", MSS, 6, mA[:])
        colsum("m2", M2S, 6, m2A[:])
        colsum("az", AZS, 7, azB[:])
        colsum("ms", MSS, 7, mB[:])
        colsum("m2", M2S, 7, m2B[:])

        zx = tl("zx", 1024)
        zy = tl("zy", 1024)
        zz = tl("zz", 1024)

        def zpieces(i, zt, final=False):
            azt = tl(f"azt{i}", 1024)
            mt = tl(f"mt{i}", 1024)
            m2t = tl(f"m2t{i}", 1024)
            act(azt[:], zt[:], AF.Abs)
            v.tensor_scalar(mt[:, 0:512], azt[:, 0:512],
                            C_A, 1.0 / C_A, ALU.min, ALU.mult)
            v.tensor_scalar(mt[:, 512:1024], azt[:, 512:1024],
                            C_B, 1.0 / C_B, ALU.min, ALU.mult)
            (v if i < 2 else g).tensor_mul(m2t[:], mt[:], mt[:])
            colsum("az", AZS, 2 * i, azt[:, 0:512])
            colsum("ms", MSS, 2 * i, mt[:, 0:512])
            colsum("m2", M2S, 2 * i, m2t[:, 0:512])
            colsum("az", AZS, 2 * i + 1, azt[:, 512:1024], final=final)
            colsum("ms", MSS, 2 * i + 1, mt[:, 512:1024], final=final)
            colsum("m2", M2S, 2 * i + 1, m2t[:, 512:1024], final=final)

        # vx = p(0,1) - p(1,0) - p(2,3) + p(3,2); z/pieces chase each comp
        q0 = prod("q0", v, 0, 1)
        q1 = prod("q1", g, 1, 0)
        q2 = prod("q2", g, 2, 3)
        q3 = prod("q3", g, 3, 2)
        va_ = tl("va_", 1024)
        vb_ = tl("vb_", 1024)
        v.tensor_sub(va_[:], q0[:], q1[:])
        g.tensor_sub(vb_[:], q3[:], q2[:])
        v.tensor_add(vx[:], va_[:], vb_[:])
        v.tensor_mul(zx[:], gp2[:], vx[:])
        zpieces(0, zx)
        # vy = p(0,2) - p(2,0) - p(3,1) + p(1,3)
        r0 = prod("r0", v, 0, 2)
        r1 = prod("r1", g, 2, 0)
        r2 = prod("r2", g, 3, 1)
        r3 = prod("r3", g, 1, 3)
        ya_ = tl("ya_", 1024)
        yb_ = tl("yb_", 1024)
        v.tensor_sub(ya_[:], r0[:], r1[:])
        g.tensor_sub(yb_[:], r3[:], r2[:])
        v.tensor_add(vy[:], ya_[:], yb_[:])
        v.tensor_mul(zy[:], gp2[:], vy[:])
        zpieces(1, zy)
        # vz = p(0,3) - p(3,0) - p(1,2) + p(2,1)
        s0 = prod("s0", v, 0, 3)
        s1 = prod("s1", g, 3, 0)
        s2 = prod("s2", g, 1, 2)
        s3 = prod("s3", g, 2, 1)
        za_ = tl("za_", 1024)
        zb_ = tl("zb_", 1024)
        v.tensor_sub(za_[:], s0[:], s1[:])
        g.tensor_sub(zb_[:], s3[:], s2[:])
        v.tensor_add(vz[:], za_[:], zb_[:])
        g.tensor_mul(zz[:], gp2[:], vz[:])
        zpieces(2, zz, final=True)

        OUTS = pool.tile([8, 1536], F32, name="OUTS", tag="OUTS")
        v.tensor_copy(OUTS[:, 0:512], AZS[:])
        dma(out_all[:, 0:512], OUTS[:, 0:512])
        v.tensor_copy(OUTS[:, 512:1024], MSS[:])
        dma(out_all[:, 512:1024], OUTS[:, 512:1024])
        v.tensor_copy(OUTS[:, 1024:1536], M2S[:])
        dma(out_all[:, 1024:1536], OUTS[:, 1024:1536])
    return nc


def _split_multi_waits(bir_json):
    import orjson
    bir = orjson.loads(bir_json)
    ctr = [0]

    def fix_block(blk):
        out = []
        for ins in blk.get("instructions", []):
            si = ins.get("sync_info") or {}
            waits = si.get("on_wait") or []
            if len(waits) > 1:
                for w in waits[:-1]:
                    ctr[0] += 1
                    out.append({
                        "debug": ins.get("debug", 0),
                        "engine": ins["engine"],
                        "ins": [], "outs": [],
                        "name": f"NWT-{ctr[0]}",
                        "opcode": "EventSemaphore",
                        "sync_info": {"on_wait": [w], "on_update": []},
                    })
                si["on_wait"] = [waits[-1]]
            out.append(ins)
        blk["instructions"] = out

    def walk(o):
        if isinstance(o, dict):
            if "instructions" in o:
                fix_block(o)
            for val in o.values():
                walk(val)
        elif isinstance(o, list):
            for val in o:
                walk(val)

    walk(bir)
    return orjson.dumps(bir)


def _install_compile_patch():
    import concourse.bass_utils as bu
    if getattr(bu, "_gyro_patched", False):
        return
    orig = bu.compile_bir_kernel

    def patched(bir_json, tmpdir, neff_name="file.neff"):
        return orig(_split_multi_waits(bir_json), tmpdir, neff_name)

    bu.compile_bir_kernel = patched
    bu._gyro_patched = True
    try:
        import concourse.bass2jax as b2j
        b2j.compile_bir_kernel = patched
    except Exception:
        pass


def _get_module():
    _install_compile_patch()
    if "nc" not in _CACHED:
        _CACHED["nc"] = _build_module()
    return _CACHED["nc"]


def _prep_core(xs_c, hat_c):
    """(8,8192,9),(8,8192,15) -> (128, 12288) fp16 partition-major planes."""
    xs_c = xs_c.copy()
    hat_c = hat_c.copy()
    xs_c[:, :N0, :] = 0.0
    hat_c[:, :N0, :] = 0.0
    xs_p = np.ascontiguousarray(xs_c.reshape(-1, 9).T)
    hat_p = np.ascontiguousarray(hat_c.reshape(-1, 15).T)
    ch = np.empty((24, 65536), np.float32)
    for k in range(3):
        ch[3 * k + 0] = xs_p[k]
        ch[3 * k + 1] = hat_p[k]
        ch[3 * k + 2] = hat_p[6 + k]
    ch[9:15] = xs_p[3:9]
    ch[15:18] = hat_p[3:6]
    ch[18:24] = hat_p[9:15]
    arr = ch.reshape(24, 128, 512).transpose(1, 0, 2).reshape(128, 12288)
    return {"planes": np.ascontiguousarray(arr).astype(np.float16)}


def _combine(res_list):
    """rows: 0 zxA 1 zxB 2 zyA 3 zyB 4 zzA 5 zzB 6 DA 7 DB."""
    total = 0.0
    groups = [(0, S_A), (1, S_B), (2, S_A), (3, S_B),
              (4, S_A), (5, S_B), (6, S_A), (7, S_B)]
    for res in res_list:
        allc = res["out_all"].astype(np.float64)
        az = allc[:, 0:512]
        ms = allc[:, 512:1024]
        m2 = allc[:, 1024:1536]
        for row, sc in groups:
            total += (0.5 * m2[row].sum() + sc * az[row].sum()
                      - ms[row].sum())
    return np.float32(W_LOSS * HUBER * HUBER * total / COUNT)


def kernel(xs, hat_xs):
    nc = _get_module()
    wpc = NW // N_CORES
    in_maps = [
        _prep_core(xs[c * wpc:(c + 1) * wpc], hat_xs[c * wpc:(c + 1) * wpc])
        for c in range(N_CORES)
    ]
    res = run_bass_kernel_spmd(nc, in_maps, list(range(N_CORES)))
    return _combine([res.results[c] for c in range(N_CORES)])
